# revision 1
# baseline (speedup 1.0000x reference)
"""Multi-head self-attention (B=8, S=1024, D=1024, H=16) on 8 trn2 cores.

Sharding: pure data-parallel over batch (B=8 -> 1 batch per core, no
collectives). Per-core kernel computes the full attention for one batch.

Layout strategy (all matmuls in float32r = 1 cycle/row):
  - Host pre-transposes activations and weights so every matmul operand is
    already in its natural [K-on-partitions, free] layout:
      xqT/xkT/xvT = x.T           [d, s]
      wqT/wkT/wvT/woT = w.T       [d, e]  (wq additionally scaled by 1/8 to
                                           fold the 1/sqrt(dk) score scale)
  - QT[e,s], KT[e,s] computed transposed (head dim on partitions); V[s,e]
    computed natural, stored with a 65th "ones" column per head so the
    attn@V matmul also produces the softmax denominator for free.
  - scoresT[k,q] per head via row-tiled dual matmuls (two heads of one
    128-partition tile run concurrently on disjoint PE row groups, K=64).
  - exp() on ScalarE directly from PSUM (no max-subtraction: |scores|<~3
    for these inputs, exp is fp32-safe).
  - attn@V: lhsT = V'[128k, 65] (64 V cols + ones), rhs = expT[128k, 512q],
    accumulated over the 8 k-tiles in PSUM; row 64 = sumexp.
  - normalize: reciprocal of row 64 (DVE), partition-broadcast via DMA,
    multiply; odd heads DMA-shifted to partitions 64..127 so OT tiles have
    the exact [e, s] layout the output projection wants as lhsT.
"""

import sys

for _p in ("/opt/trn_rl_repo", "/root/.axon_site/_ro/trn_rl_repo"):
    if _p not in sys.path:
        sys.path.append(_p)

import numpy as np

import concourse.bass as bass
import concourse.mybir as mybir
import concourse.tile as tile
from concourse import bacc
from concourse.bass_utils import run_bass_kernel_spmd

F32 = mybir.dt.float32
F32R = mybir.dt.float32r
EXP = mybir.ActivationFunctionType.Exp

S = 1024   # sequence length
D = 1024   # model dim
H = 16     # heads
DK = 64    # head dim
P = 128    # partitions
QC = 512   # q-chunk (psum bank free size in fp32)
NT = D // P   # 8 e-tiles / d-tiles / s-tiles
NB = 8     # batches == cores

VW = DK + 1  # 65: V columns per head incl. ones column


def _emit(tc, io, phases="ABCDE"):
    nc = tc.nc

    const = tc.alloc_tile_pool(name="const", bufs=1)
    persist = tc.alloc_tile_pool(name="persist", bufs=1)
    stream = tc.alloc_tile_pool(name="stream", bufs=1)

    # --- constants -------------------------------------------------------
    # per-partition bias columns: column t holds bias[t*128:(t+1)*128]
    bq_sb = const.tile([P, NT], F32, tag="bq", name="bq_sb")
    nc.sync.dma_start(bq_sb[:], io["bqs"].ap().rearrange("(t p) -> p t", p=P))
    bk_sb = const.tile([P, NT], F32, tag="bk", name="bk_sb")
    nc.sync.dma_start(bk_sb[:], io["bk"].ap().rearrange("(t p) -> p t", p=P))
    # row-broadcast bias tiles [128, D] for biases added along the free dim
    bv_bc = const.tile([P, D], F32, tag="bv", name="bv_bc")
    nc.sync.dma_start(
        bv_bc[:].unsqueeze(1), io["bv"].ap().unsqueeze(0).partition_broadcast(P)
    )
    bo_bc = const.tile([P, D], F32, tag="bo", name="bo_bc")
    nc.sync.dma_start(
        bo_bc[:].unsqueeze(1), io["bo"].ap().unsqueeze(0).partition_broadcast(P)
    )
    # all-ones [128, 64] tile: lhsT for the rank-1 reciprocal broadcast
    ones_sb = const.tile([P, DK], F32R, tag="ones", name="ones_sb")
    nc.sync.dma_start(
        ones_sb[:].unsqueeze(1), io["onesd"].ap().unsqueeze(0).partition_broadcast(P)
    )

    # --- persistent SBUF tensors ----------------------------------------
    QT = [persist.tile([P, S], F32R, tag=f"qt{t}", name=f"qt{t}") for t in range(NT)]
    KT = [persist.tile([P, S], F32R, tag=f"kt{t}", name=f"kt{t}") for t in range(NT)]
    # V with a ones column appended per head: [s, 16*65]
    V = [persist.tile([P, H * VW], F32R, tag=f"v{t}", name=f"v{t}") for t in range(NT)]
    OT = [persist.tile([P, S], F32R, tag=f"ot{t}", name=f"ot{t}") for t in range(NT)]

    # ones columns of V (column 64 of each head's 65-wide group)
    for st in range(NT):
        v_view = V[st][:].rearrange("p (h k) -> p h k", k=VW)
        nc.sync.dma_start(
            v_view[:, :, DK:VW].unsqueeze(1),
            io["onesw"].ap().unsqueeze(1).unsqueeze(0).partition_broadcast(P),
        )

    # One unified PSUM pool for every phase: 4 slots x [128, 1024] (2 banks
    # each) = all 8 banks. No pool-closure barriers between phases, so the
    # scheduler is free to overlap projections / attention / output
    # projection wherever data dependencies allow.
    upool = tc.alloc_tile_pool(name="upool", bufs=4, space="PSUM")

    def psum8(pfx):
        pairs = [
            upool.tile([P, 2 * QC], F32, tag="u", name=f"{pfx}_{s}")
            for s in range(NT // 2)
        ]
        return [pairs[t // 2][:, (t % 2) * QC : (t % 2 + 1) * QC] for t in range(NT)]

    # --- phases A/B: QT / KT projections (transposed, bias per-partition)
    if "A" in phases:
        for dst, wname, xname, bias in (
            (QT, "wqT", "xqT", bq_sb),
            (KT, "wkT", "xkT", bk_sb),
        ):
            w_ap = io[wname].ap()
            x_ap = io[xname].ap()
            for c in range(2):
                ps = psum8(f"ps_{wname}{c}")
                for d in range(NT):
                    xt = stream.tile(
                        [P, QC], F32R, tag="xc", bufs=3, name=f"x_{xname}{c}_{d}"
                    )
                    nc.sync.dma_start(
                        xt[:], x_ap[d * P : (d + 1) * P, c * QC : (c + 1) * QC]
                    )
                    wt = stream.tile(
                        [P, D], F32R, tag="big", bufs=3, name=f"w_{wname}{c}_{d}"
                    )
                    nc.sync.dma_start(wt[:], w_ap[d * P : (d + 1) * P, :])
                    for t in range(NT):
                        nc.tensor.matmul(
                            ps[t][:],
                            lhsT=wt[:, t * P : (t + 1) * P],
                            rhs=xt[:],
                            start=(d == 0),
                            stop=(d == NT - 1),
                        )
                for t in range(NT):
                    nc.vector.tensor_scalar_add(
                        dst[t][:, c * QC : (c + 1) * QC], ps[t][:], bias[:, t : t + 1]
                    )

    if "C" in phases:
        # --- phase C: V projection (natural layout, strided into 65-wide
        # head groups, bias broadcast along free dim)
        xv_ap = io["xvT"].ap()
        wv_ap = io["wvT"].ap()
        for c in range(2):
            ps = psum8(f"ps_v{c}")
            for d in range(NT):
                xb = stream.tile([P, D], F32R, tag="big", bufs=3, name=f"x_v{c}_{d}")
                nc.sync.dma_start(xb[:], xv_ap[d * P : (d + 1) * P, :])
                wc = stream.tile([P, QC], F32R, tag="xc", bufs=3, name=f"w_v{c}_{d}")
                nc.sync.dma_start(
                    wc[:], wv_ap[d * P : (d + 1) * P, c * QC : (c + 1) * QC]
                )
                for st in range(NT):
                    nc.tensor.matmul(
                        ps[st][:],
                        lhsT=xb[:, st * P : (st + 1) * P],
                        rhs=wc[:],
                        start=(d == 0),
                        stop=(d == NT - 1),
                    )
            for st in range(NT):
                v_out = V[st][:].rearrange("p (h k) -> p h k", k=VW)[
                    :, 8 * c : 8 * c + 8, 0:DK
                ]
                ps_v = ps[st][:].rearrange("p (h k) -> p h k", k=DK)
                bv_v = bv_bc[:, c * QC : (c + 1) * QC].rearrange(
                    "p (h k) -> p h k", k=DK
                )
                nc.vector.tensor_add(v_out, ps_v, bv_v)

    # --- phase D: attention, one head-pair (= one 128-row e-tile) at a time
    if "D" in phases:
      with tc.tile_pool(name="dsb", bufs=1) as dsb:
        for p in range(NT):
            he, ho = 2 * p, 2 * p + 1
            for qi in range(2):
                qs = slice(qi * QC, (qi + 1) * QC)
                # AV accumulator first so it grabs a slot before the
                # score tiles start rotating through the remaining three.
                av = upool.tile([P, 2 * QC], F32, tag="u", name=f"av{p}_{qi}")
                ave = av[:, 0:QC]
                avo = av[:, QC : 2 * QC]
                ats = []
                for g in range(4):  # groups of 2 k-blocks -> [128, 1024] psum
                    sce = upool.tile(
                        [P, 2 * QC], F32, tag="u", name=f"sce{p}_{qi}_{g}"
                    )
                    sco = upool.tile(
                        [P, 2 * QC], F32, tag="u", name=f"sco{p}_{qi}_{g}"
                    )
                    for j in range(2):
                        kb = 2 * g + j
                        ksl = slice(kb * P, (kb + 1) * P)
                        nc.tensor.matmul(
                            sce[:, j * QC : (j + 1) * QC],
                            lhsT=KT[p][0:64, ksl],
                            rhs=QT[p][0:64, qs],
                            start=True,
                            stop=True,
                        )
                        nc.tensor.matmul(
                            sco[:, j * QC : (j + 1) * QC],
                            lhsT=KT[p][64:128, ksl],
                            rhs=QT[p][64:128, qs],
                            start=True,
                            stop=True,
                            tile_position=(64, 0),
                        )
                    ae = dsb.tile(
                        [P, 2 * QC], F32R, tag="at", bufs=8, name=f"ae{p}_{qi}_{g}"
                    )
                    nc.scalar.activation(ae[:], sce[:], EXP)
                    ao = dsb.tile(
                        [P, 2 * QC], F32R, tag="at", bufs=8, name=f"ao{p}_{qi}_{g}"
                    )
                    nc.scalar.activation(ao[:], sco[:], EXP)
                    ats.append((ae, ao))

                for g in range(4):
                    ae, ao = ats[g]
                    for j in range(2):
                        kb = 2 * g + j
                        nc.tensor.matmul(
                            ave[0 : VW, :],
                            lhsT=V[kb][:, he * VW : (he + 1) * VW],
                            rhs=ae[:, j * QC : (j + 1) * QC],
                            start=(kb == 0),
                            stop=(kb == NT - 1),
                        )
                        nc.tensor.matmul(
                            avo[0 : VW, :],
                            lhsT=V[kb][:, ho * VW : (ho + 1) * VW],
                            rhs=ao[:, j * QC : (j + 1) * QC],
                            start=(kb == 0),
                            stop=(kb == NT - 1),
                        )

                # normalize: reciprocal of sumexp rows (partition 64),
                # rank-1 PE broadcast down to partitions 0..63, then multiply
                rece = dsb.tile([P, QC], F32R, tag="rec", bufs=2, name=f"rece{p}_{qi}")
                reco = dsb.tile([P, QC], F32R, tag="rec", bufs=2, name=f"reco{p}_{qi}")
                with nc.allow_low_precision(reason="f32r is fp32 bits"):
                    nc.vector.reciprocal(rece[64:65, :], ave[64:65, :])
                    nc.vector.reciprocal(reco[64:65, :], avo[64:65, :])
                bc = upool.tile([P, 2 * QC], F32, tag="u", name=f"bc{p}_{qi}")
                nc.tensor.matmul(
                    bc[0:DK, 0:QC], lhsT=ones_sb[64:65, :], rhs=rece[64:65, :],
                    start=True, stop=True, tile_position=(64, 0),
                )
                nc.tensor.matmul(
                    bc[0:DK, QC : 2 * QC], lhsT=ones_sb[64:65, :], rhs=reco[64:65, :],
                    start=True, stop=True, tile_position=(64, 0),
                )
                rbc = dsb.tile([DK, 2 * QC], F32, tag="rbc", bufs=2, name=f"rbc{p}_{qi}")
                nc.vector.tensor_copy(rbc[:], bc[0:DK, :])
                nc.vector.tensor_mul(OT[p][0:64, qs], ave[0:64, :], rbc[:, 0:QC])
                tmpo = dsb.tile([64, QC], F32R, tag="tmp", bufs=2, name=f"tmpo{p}_{qi}")
                nc.vector.tensor_mul(tmpo[:], avo[0:64, :], rbc[:, QC : 2 * QC])
                nc.sync.dma_start(OT[p][64:128, qs], tmpo[:])

    # --- phase E: output projection out[s, f] = OT.T @ woT + bo ----------
    out_ap = io["out"].ap()
    wo_ap = io["woT"].ap()
    if "E" in phases:
      with tc.tile_pool(name="esb", bufs=1) as esb:
        for c in range(2):
            fs = slice(c * QC, (c + 1) * QC)
            ps = psum8(f"ps_o{c}")
            for e in range(NT):
                wt = stream.tile([P, QC], F32R, tag="xc", bufs=3, name=f"w_o{c}_{e}")
                nc.sync.dma_start(wt[:], wo_ap[e * P : (e + 1) * P, fs])
                for st in range(NT):
                    nc.tensor.matmul(
                        ps[st][:],
                        lhsT=OT[e][:, st * P : (st + 1) * P],
                        rhs=wt[:],
                        start=(e == 0),
                        stop=(e == NT - 1),
                    )
            for st in range(NT):
                ob = esb.tile([P, QC], F32, tag="ob", bufs=3, name=f"ob{c}_{st}")
                nc.vector.tensor_add(ob[:], ps[st][:], bo_bc[:, fs])
                nc.sync.dma_start(out_ap[st * P : (st + 1) * P, fs], ob[:])

    if "E" not in phases:
        # bench-only: drain something comparable to E's output traffic
        srcs = OT if "D" in phases else QT
        for t in range(NT):
            nc.sync.dma_start(out_ap[t * P : (t + 1) * P, :].bitcast(F32R), srcs[t][:, 0:S])

    upool.release()
    stream.release()
    persist.release()
    const.release()


def build_nc(repeats=1, phases="ABCDE"):
    nc = bacc.Bacc(
        "TRN2",
        target_bir_lowering=False,
        debug=False,
        enable_asserts=False,
        num_devices=NB,
    )
    io = {}
    for name in ("xqT", "xkT", "xvT"):
        io[name] = nc.dram_tensor(name, [D, S], F32R, kind="ExternalInput")
    for name in ("wqT", "wkT", "wvT", "woT"):
        io[name] = nc.dram_tensor(name, [D, D], F32R, kind="ExternalInput")
    for name in ("bqs", "bk", "bv", "bo"):
        io[name] = nc.dram_tensor(name, [D], F32, kind="ExternalInput")
    io["onesw"] = nc.dram_tensor("onesw", [H], F32R, kind="ExternalInput")
    io["onesd"] = nc.dram_tensor("onesd", [DK], F32R, kind="ExternalInput")
    io["out"] = nc.dram_tensor("out", [S, D], F32, kind="ExternalOutput")

    with tile.TileContext(nc) as tc:
        for _ in range(repeats):
            _emit(tc, io, phases)
    nc.compile()
    return nc


_CACHE = {}


def get_nc():
    if "nc" not in _CACHE:
        _CACHE["nc"] = build_nc()
    return _CACHE["nc"]


def make_in_maps(query, key, value, wq, bq, wk, bk, wv, bv, wo, bo):
    f = np.float32
    # fold the 1/sqrt(DK) score scaling into the Q projection (exact: 1/8)
    wqT = np.ascontiguousarray(np.asarray(wq, f).T) * f(0.125)
    bqs = np.asarray(bq, f) * f(0.125)
    wkT = np.ascontiguousarray(np.asarray(wk, f).T)
    wvT = np.ascontiguousarray(np.asarray(wv, f).T)
    woT = np.ascontiguousarray(np.asarray(wo, f).T)
    common = {
        "wqT": wqT, "wkT": wkT, "wvT": wvT, "woT": woT,
        "bqs": np.ascontiguousarray(bqs),
        "bk": np.ascontiguousarray(np.asarray(bk, f)),
        "bv": np.ascontiguousarray(np.asarray(bv, f)),
        "bo": np.ascontiguousarray(np.asarray(bo, f)),
        "onesw": np.ones(H, f),
        "onesd": np.ones(DK, f),
    }
    q = np.asarray(query, f)
    k = np.asarray(key, f)
    v = np.asarray(value, f)
    in_maps = []
    for b in range(NB):
        in_maps.append(
            {
                "xqT": np.ascontiguousarray(q[b].T),
                "xkT": np.ascontiguousarray(k[b].T),
                "xvT": np.ascontiguousarray(v[b].T),
                **common,
            }
        )
    return in_maps


def kernel(
    query,
    key,
    value,
    inputs_attn_mask=None,  # all-ones per spec; masking is a no-op
    wq=None, bq=None, wk=None, bk=None, wv=None, bv=None, wo=None, bo=None,
    **_extra,
):
    nc = get_nc()
    in_maps = make_in_maps(query, key, value, wq, bq, wk, bk, wv, bv, wo, bo)
    res = run_bass_kernel_spmd(nc, in_maps, core_ids=list(range(NB)))
    out = np.stack([res.results[b]["out"] for b in range(NB)], axis=0)
    return out.astype(np.float32)



# revision 33
# speedup vs baseline: 21.0810x; 21.0810x over previous
"""Multi-head self-attention (B=8, S=1024, D=1024, H=16) on 8 trn2 cores.

Sharding: pure data-parallel over batch (B=8 -> 1 batch per core, no
collectives). Per-core kernel computes the full attention for one batch.

v3 design (v2 at ~328us, v1 baseline at ~670us):
  - all matmul operands bf16 (halves DMA bytes, enables FWL weight loads so
    LDWEIGHTS hides behind matmuls; PSUM accumulation stays fp32)
  - weights + transposed activations resident in SBUF, loaded with large
    contiguous DMAs split across the two HWDGE rings (sync + scalar),
    interleaved so the first projection matmuls can start ASAP
  - warm-up matmuls on constant data during the initial DMA fill keep the
    PE HAM clock-gate at full rate for the first real matmuls
  - V bias folded into the output bias on host (bo' = bo + wo @ bv); output
    bias applied via a K=1 ones-row matmul accumulated into the E psum
  - per-head-pair software pipeline, depth 1: block u's attn@V matmuls are
    interleaved into block u+1's score stream so the PE stays dense while
    ScalarE exps gate the score rate (avoids HAM re-throttle per pair)
  - psum: 2 rotating score slots + 1 dedicated projection slot + 1 attn
    accumulator slot (8 banks total) so projections never starve scores
  - softmax denominator via 65th ones column of V (psum row 64); cross-base
    DVE copy to partition 0, fast custom-DVE reciprocal, gpsimd partition
    broadcast; odd head normalizes straight into OT rows 64:127 via a
    cross-base tensor_mul
"""

import sys

for _p in ("/opt/trn_rl_repo", "/root/.axon_site/_ro/trn_rl_repo"):
    if _p not in sys.path:
        sys.path.append(_p)

import numpy as np
import ml_dtypes

import concourse.bass as bass
import concourse.mybir as mybir
import concourse.tile as tile
from concourse import bacc
from concourse.bass_utils import run_bass_kernel_spmd

F32 = mybir.dt.float32
BF16 = mybir.dt.bfloat16
FP8 = mybir.dt.float8e4
DR = mybir.MatmulPerfMode.DoubleRow
EXP = mybir.ActivationFunctionType.Exp
COPY = mybir.ActivationFunctionType.Copy

S = 1024   # sequence length
D = 1024   # model dim
H = 16     # heads
DK = 64    # head dim
P = 128    # partitions
QC = 512   # q-chunk / psum bank free size in fp32
NT = D // P   # 8 e-tiles / d-tiles / s-tiles
NB = 8     # batches == cores

VW = DK + 1  # 65: V columns per head incl. ones column
NWARM = 20  # warm-up matmuls during initial DMA fill
# Q/K weights are stored fp8 scaled by 8 (dodges the fp8 denormal range);
# the combined 1/(8*8*sqrt(dk)) = 1/512 score scale is folded into the exp.
QK_SCALE = 8.0
EXP_SCALE = 0.125 / (QK_SCALE * QK_SCALE)


def _emit(tc, io):
    nc = tc.nc

    const = tc.alloc_tile_pool(name="const", bufs=1)
    wpool = tc.alloc_tile_pool(name="wpool", bufs=1)
    vwpool = tc.alloc_tile_pool(name="vwpool", bufs=1, side="right")
    persist = tc.alloc_tile_pool(name="persist", bufs=1)

    # --- constants -------------------------------------------------------
    bq_sb = const.tile([P, NT], F32, tag="bq", name="bq_sb")
    nc.sync.dma_start(bq_sb[:], io["bqs"].ap().rearrange("(t p) -> p t", p=P))
    bk_sb = const.tile([P, NT], F32, tag="bk", name="bk_sb")
    nc.sync.dma_start(bk_sb[:], io["bk"].ap().rearrange("(t p) -> p t", p=P))
    # adjusted output bias row (bo + wo @ bv), single partition
    bo_sb = const.tile([1, D], BF16, tag="bo", name="bo_sb")
    nc.sync.dma_start(bo_sb[:], io["bo_row"].ap().unsqueeze(0))
    # ones row for the K=1 bias matmul in phase E (and warm-up matmuls)
    ones1 = const.tile([1, P], BF16, tag="ones1", name="ones1")
    nc.gpsimd.memset(ones1[:], 1.0)
    # warm-up rhs with no DMA dependency
    ones2 = const.tile([1, QC], BF16, tag="ones2", name="ones2")
    nc.gpsimd.memset(ones2[:], 1.0)

    # --- resident weights / activations (bf16) ---------------------------
    # sync ring: x tensors; scalar ring: weights (both HWDGE), interleaved
    # per-d so the first projection matmuls can begin after two transfers.
    XV = [vwpool.tile([P, S], BF16, tag=f"xv{d}", name=f"xv{d}") for d in range(NT)]
    WV = [vwpool.tile([P, D], BF16, tag=f"wv{d}", name=f"wv{d}") for d in range(NT)]
    # fp8 Q/K operands in DoubleRow layout: [128 partitions, NT k-subtiles, S]
    XQ8 = wpool.tile([P, NT * S], FP8, tag="xq8", name="xq8")
    XK8 = wpool.tile([P, NT * S], FP8, tag="xk8", name="xk8")
    WQ8 = wpool.tile([P, NT * D], FP8, tag="wq8", name="wq8")
    WK8 = wpool.tile([P, NT * D], FP8, tag="wk8", name="wk8")
    WO = [wpool.tile([P, D], BF16, tag=f"wo{d}", name=f"wo{d}") for d in range(NT)]
    nc.sync.dma_start(XQ8[:], io["xq8"].ap())
    nc.scalar.dma_start(WQ8[:], io["wq8"].ap())
    nc.sync.dma_start(XK8[:], io["xk8"].ap())
    nc.scalar.dma_start(WK8[:], io["wk8"].ap())
    for d in range(NT):
        nc.sync.dma_start(XV[d][:], io["xvT"].ap()[d * P : (d + 1) * P, :])
        nc.scalar.dma_start(WV[d][:], io["wvT"].ap()[d * P : (d + 1) * P, :])
    for d in range(NT):
        nc.scalar.dma_start(WO[d][:], io["woT"].ap()[d * P : (d + 1) * P, :])
    xq8v = XQ8[:].rearrange("p (k s) -> p k s", s=S)
    xk8v = XK8[:].rearrange("p (k s) -> p k s", s=S)
    wq8v = WQ8[:].rearrange("p (k e) -> p k e", e=D)
    wk8v = WK8[:].rearrange("p (k e) -> p k e", e=D)

    # --- persistent SBUF tensors ----------------------------------------
    QT = [persist.tile([P, S], BF16, tag=f"qt{t}", name=f"qt{t}") for t in range(NT)]
    KT = [persist.tile([P, S], BF16, tag=f"kt{t}", name=f"kt{t}") for t in range(NT)]
    V = [persist.tile([P, H * VW], BF16, tag=f"v{t}", name=f"v{t}") for t in range(NT)]
    OT = [persist.tile([P, S], BF16, tag=f"ot{t}", name=f"ot{t}") for t in range(NT)]

    for st in range(NT):
        v_view = V[st][:].rearrange("p (h k) -> p h k", k=VW)
        nc.gpsimd.memset(v_view[:, :, DK:VW], 1.0)

    # PSUM: 2 rotating score slots (4 banks) + dedicated projection slot
    # (2 banks) + attn accumulator slot (2 banks) = 8 banks
    scpool = tc.alloc_tile_pool(name="scpool", bufs=2, space="PSUM")
    pjpool = tc.alloc_tile_pool(name="pjpool", bufs=1, space="PSUM")
    avpool = tc.alloc_tile_pool(name="avpool", bufs=1, space="PSUM")

    vwpool.release()
    work = tc.alloc_tile_pool(name="work", bufs=1)

    # --- warm-up matmuls: keep the PE busy (and the HAM un-throttled)
    # while the first input DMAs stream in. K=1 rank-1 fills, no consumer,
    # no DMA dependencies (both operands are memsets).
    wu = avpool.tile([P, 2 * QC], F32, tag="av", name="warmup")
    for i in range(NWARM):
        nc.tensor.matmul(
            wu[:, 0:QC] if i % 2 == 0 else wu[:, QC : 2 * QC],
            lhsT=ones1[0:1, :],
            rhs=ones2[0:1, :],
            start=True,
            stop=True,
        )

    # --- V projection ----------------------------------------------------
    def v_proj_group(c, stg, pool_, tag):
        cs = slice(c * QC, (c + 1) * QC)
        ps = pool_.tile([P, 2 * QC], F32, tag=tag, name=f"vps{c}_{stg}")
        for d in range(NT):
            for h in range(2):
                st = 2 * stg + h
                nc.tensor.matmul(
                    ps[:, h * QC : (h + 1) * QC],
                    lhsT=XV[d][:, st * P : (st + 1) * P],
                    rhs=WV[d][:, cs],
                    start=(d == 0),
                    stop=(d == NT - 1),
                )
        for h in range(2):
            st = 2 * stg + h
            dst = V[st][:].rearrange("p (h k) -> p h k", k=VW)[
                :, 8 * c : 8 * c + 8, 0:DK
            ]
            src = ps[:, h * QC : (h + 1) * QC].rearrange("p (h k) -> p h k", k=DK)
            nc.vector.tensor_copy(dst, src)

    # heads 0-7 (c=0) run right after pair 0's projection (so pair 0's bias
    # adds hit the DVE queue first); heads 8-15 (c=1, first needed by pair
    # 4) are deferred and interleaved into pair 0's exp-gated ramp
    vslots = [scpool, scpool, pjpool]
    vc1_groups = list(range(NT // 2))  # c=1 groups, drained in block (0,0)

    # --- per-head-pair pipeline with depth-1 attn@V software pipeline ----
    def qk_one_proj(p, w8v, x8v, bias_sb, dstT, nm):
        psl = slice(p * P, (p + 1) * P)
        ps = pjpool.tile([P, 2 * QC], F32, tag="pj", name=f"{nm}ps{p}")
        for ks in range(NT // 2):
            kk = slice(2 * ks, 2 * ks + 2)
            for c in range(2):
                nc.tensor.matmul(
                    ps[:, c * QC : (c + 1) * QC],
                    lhsT=w8v[:, kk, psl],
                    rhs=x8v[:, kk, c * QC : (c + 1) * QC],
                    start=(ks == 0),
                    stop=(ks == NT // 2 - 1),
                    perf_mode=DR,
                )
        for c in range(2):
            nc.vector.tensor_scalar_add(
                dstT[p][:, c * QC : (c + 1) * QC],
                ps[:, c * QC : (c + 1) * QC],
                bias_sb[:, p : p + 1],
            )

    def av_mms(ctx, j):
        av, exs, p, qi = ctx
        he, ho = 2 * p, 2 * p + 1
        ex = exs[j]
        assert av is not None
        nc.tensor.matmul(
            av[0:VW, 0:QC],
            lhsT=V[j][:, he * VW : (he + 1) * VW],
            rhs=ex[:, 0:QC],
            start=(j == 0),
            stop=(j == NT - 1),
        )
        nc.tensor.matmul(
            av[0:VW, QC : 2 * QC],
            lhsT=V[j][:, ho * VW : (ho + 1) * VW],
            rhs=ex[:, QC : 2 * QC],
            start=(j == 0),
            stop=(j == NT - 1),
        )

    def normalize(ctx):
        av, exs, p, qi = ctx
        qs = slice(qi * QC, (qi + 1) * QC)
        den = work.tile([1, 2 * QC], F32, tag="den", bufs=2, name=f"den{p}_{qi}")
        nc.vector.tensor_copy(den[0:1, :], av[64:65, :])
        rec = work.tile([1, 2 * QC], F32, tag="rec", bufs=2, name=f"rec{p}_{qi}")
        nc.vector.reciprocal_approx_fast(rec[0:1, :], den[0:1, :])
        rbc = work.tile([DK, 2 * QC], F32, tag="rbc", bufs=2, name=f"rbc{p}_{qi}")
        nc.gpsimd.partition_broadcast(rbc[:], rec[0:1, :], channels=DK)
        nc.vector.tensor_mul(OT[p][0:64, qs], av[0:64, 0:QC], rbc[:, 0:QC])
        nc.vector.tensor_mul(
            OT[p][64:128, qs], av[0:64, QC : 2 * QC], rbc[:, QC : 2 * QC]
        )

    # interleave schedule: cumulative count of prev-block attn@V matmul
    # PAIRS emitted by the end of each kb's scores. Blocks right after a
    # projection have 4 pairs pre-drained into the projection seams. The
    # chain finishes by kb 2-3 so normalize() (whose muls free the attn
    # accumulator psum slot) completes well before the next block needs it.
    AV_SCHED_Q0 = [4, 6, 8, 8, 8, 8, 8, 8]
    AV_SCHED_Q1 = [2, 4, 6, 8, 8, 8, 8, 8]

    prev = None  # [av_tile_or_None, exs, p, qi]
    emitted = 0

    def drain_av(upto):
        nonlocal emitted
        while prev is not None and emitted < upto:
            if prev[0] is None:
                prev[0] = avpool.tile(
                    [P, 2 * QC], F32, tag="av", name=f"av{prev[2]}_{prev[3]}"
                )
            av_mms(prev, emitted)
            emitted += 1

    # where the deferred c=1 V-projection groups get emitted: (p, qi, kb)
    VC1_SLOTS = {(0, 0, 0), (0, 0, 4), (0, 1, 5), (1, 0, 2)}

    for p in range(NT):
        qk_one_proj(p, wq8v, xq8v, bq_sb, QT, "q")
        drain_av(2)
        qk_one_proj(p, wk8v, xk8v, bk_sb, KT, "k")
        drain_av(4)
        if p == 0:
            for stg in range(NT // 2):
                pool_ = vslots[stg % 3]
                v_proj_group(0, stg, pool_, "sc" if pool_ is scpool else "pj")
        for qi in range(2):
            sched = AV_SCHED_Q0 if qi == 0 else AV_SCHED_Q1
            qs = slice(qi * QC, (qi + 1) * QC)
            exs = []
            for kb in range(NT):
                # drain pipelined work BEFORE this kb's score matmuls: the
                # PE queue is in-order, and the score matmul may wait on an
                # exp slot — anything queued behind it would idle too.
                if prev is not None:
                    drain_av(sched[kb])
                if vc1_groups and (p, qi, kb) in VC1_SLOTS:
                    v_proj_group(1, vc1_groups.pop(0), pjpool, "pj")
                ksl = slice(kb * P, (kb + 1) * P)
                sc = scpool.tile([P, 2 * QC], F32, tag="sc", name=f"sc{p}_{qi}_{kb}")
                nc.tensor.matmul(
                    sc[:, 0:QC],
                    lhsT=KT[p][0:64, ksl],
                    rhs=QT[p][0:64, qs],
                    start=True,
                    stop=True,
                )
                nc.tensor.matmul(
                    sc[:, QC : 2 * QC],
                    lhsT=KT[p][64:128, ksl],
                    rhs=QT[p][64:128, qs],
                    start=True,
                    stop=True,
                    tile_position=(64, 0),
                )
                ex = work.tile(
                    [P, 2 * QC], BF16, tag="ex", bufs=12, name=f"ex{p}_{qi}_{kb}"
                )
                nc.scalar.activation(ex[:], sc[:], EXP, scale=EXP_SCALE)
                exs.append(ex)
                if prev is not None and kb == 4:
                    normalize(prev)
            prev = [None, exs, p, qi]
            emitted = 0
    # drain the last block
    drain_av(NT)
    normalize(prev)

    # --- output projection: out[s, f] = OT.T @ woT + bo' -----------------
    out_ap = io["out"].ap()
    eslots = [scpool, scpool, pjpool]
    si = 0
    for c in range(2):
        fs = slice(c * QC, (c + 1) * QC)
        for stg in range(NT // 2):
            pool_ = eslots[si % 3]
            si += 1
            ps = pool_.tile(
                [P, 2 * QC], F32,
                tag="sc" if pool_ is scpool else "pj",
                name=f"ops{c}_{stg}",
            )
            for h in range(2):
                st = 2 * stg + h
                seg = ps[:, h * QC : (h + 1) * QC]
                for e in range(NT):
                    nc.tensor.matmul(
                        seg,
                        lhsT=OT[e][:, st * P : (st + 1) * P],
                        rhs=WO[e][:, fs],
                        start=(e == 0),
                        stop=False,
                    )
                nc.tensor.matmul(
                    seg,
                    lhsT=ones1[0:1, :],
                    rhs=bo_sb[0:1, fs],
                    start=False,
                    stop=True,
                )
            for h in range(2):
                st = 2 * stg + h
                ob = work.tile([P, QC], F32, tag="ob", bufs=4, name=f"ob{c}_{stg}_{h}")
                nc.vector.tensor_copy(ob[:], ps[:, h * QC : (h + 1) * QC])
                nc.sync.dma_start(out_ap[st * P : (st + 1) * P, fs], ob[:])

    avpool.release()
    pjpool.release()
    scpool.release()
    work.release()
    persist.release()
    wpool.release()
    const.release()


def build_nc():
    nc = bacc.Bacc(
        "TRN2",
        target_bir_lowering=False,
        debug=False,
        enable_asserts=False,
        num_devices=NB,
    )
    io = {}
    io["xvT"] = nc.dram_tensor("xvT", [D, S], BF16, kind="ExternalInput")
    for name in ("wvT", "woT"):
        io[name] = nc.dram_tensor(name, [D, D], BF16, kind="ExternalInput")
    for name in ("xq8", "xk8"):
        io[name] = nc.dram_tensor(name, [P, NT * S], FP8, kind="ExternalInput")
    for name in ("wq8", "wk8"):
        io[name] = nc.dram_tensor(name, [P, NT * D], FP8, kind="ExternalInput")
    for name in ("bqs", "bk"):
        io[name] = nc.dram_tensor(name, [D], F32, kind="ExternalInput")
    io["bo_row"] = nc.dram_tensor("bo_row", [D], BF16, kind="ExternalInput")
    io["out"] = nc.dram_tensor("out", [S, D], F32, kind="ExternalOutput")

    with tile.TileContext(nc) as tc:
        _emit(tc, io)
    nc.compile()
    return nc


_CACHE = {}


def get_nc():
    if "nc" not in _CACHE:
        _CACHE["nc"] = build_nc()
    return _CACHE["nc"]


def _dr_layout(a):
    """[D, cols] -> DoubleRow fp8 layout [128, NT * cols] where partition p,
    k-subtile k holds row k*128 + p."""
    f8 = ml_dtypes.float8_e4m3
    cols = a.shape[1]
    out = a.reshape(NT, P, cols).transpose(1, 0, 2).reshape(P, NT * cols)
    return np.ascontiguousarray(out.astype(f8))


def make_in_maps(query, key, value, wq, bq, wk, bk, wv, bv, wo, bo):
    f = np.float32
    bf = ml_dtypes.bfloat16
    # Q/K weights fp8, scaled by 8 to stay clear of the fp8 denormal range;
    # the combined 1/512 score scale is applied inside the exp activation.
    wq8 = _dr_layout(np.asarray(wq, f).T * f(QK_SCALE))
    wk8 = _dr_layout(np.asarray(wk, f).T * f(QK_SCALE))
    bqs = np.asarray(bq, f) * f(QK_SCALE)
    bks = np.asarray(bk, f) * f(QK_SCALE)
    wvT = np.asarray(wv, f).T.astype(bf)
    woT = np.asarray(wo, f).T.astype(bf)
    # fold the V bias through attention into the output bias:
    # out = (attn_out + bv) @ wo.T + bo = attn_out @ wo.T + (bo + wo @ bv)
    bo_row = (np.asarray(bo, f) + np.asarray(wo, f) @ np.asarray(bv, f)).astype(bf)
    common = {
        "wq8": wq8,
        "wk8": wk8,
        "wvT": np.ascontiguousarray(wvT),
        "woT": np.ascontiguousarray(woT),
        "bqs": np.ascontiguousarray(bqs),
        "bk": np.ascontiguousarray(bks),
        "bo_row": np.ascontiguousarray(bo_row),
    }
    q = np.asarray(query, f)
    k = np.asarray(key, f)
    v = np.asarray(value, f)
    in_maps = []
    for b in range(NB):
        in_maps.append(
            {
                "xq8": _dr_layout(q[b].T),
                "xk8": _dr_layout(k[b].T),
                "xvT": np.ascontiguousarray(v[b].T.astype(bf)),
                **common,
            }
        )
    return in_maps


def kernel(
    query,
    key,
    value,
    inputs_attn_mask=None,  # all-ones per spec; masking is a no-op
    wq=None, bq=None, wk=None, bk=None, wv=None, bv=None, wo=None, bo=None,
    **_extra,
):
    nc = get_nc()
    in_maps = make_in_maps(query, key, value, wq, bq, wk, bk, wv, bv, wo, bo)
    res = run_bass_kernel_spmd(nc, in_maps, core_ids=list(range(NB)))
    out = np.stack([res.results[b]["out"] for b in range(NB)], axis=0)
    return out.astype(np.float32)
